# revision 1
# baseline (speedup 1.0000x reference)
"""MQA attention with ALiBi + causal mask on 8 TRN2 NeuronCores.

Problem: hidden_states [2,2048,4096] @ Wq -> 32 query heads of 128; single
KV head via Wkv; scores + ALiBi bias + causal mask; softmax; @ Wo.

Distribution (avoids the 64 MiB AllReduce of plain head-TP):
- Core c owns tokens [256c, 256(c+1)) of BOTH batches for all projections
  (output rows disjoint -> host concatenates). Attention is head-sharded
  round-robin (core c gets heads {c, c+8, c+16, c+24}) so every core's
  causal + ALiBi-cutoff workload is identical. The two shardings are
  bridged by small bf16 AllToAlls of qT / attnT plus AllGathers of the
  tiny single-head K/V. The qT AllToAlls are split per (batch, head-half)
  so attention starts while the q projection is still running, and
  batch-0's output projection is emission-interleaved with batch-1's
  attention so the PE-bound and DVE/ACT-bound work overlap.
- ALiBi distance cutoff: a kv chunk whose distance exceeds 45/slope
  contributes < 1e-13 of the softmax mass and is skipped. Per-slot
  (head-octile) uniform bounds keep the SPMD program identical per core.
- All matmuls in bf16 (rel-err budget 2e-2); softmax in f32 without
  max-subtraction (scores are O(10), bias <= 0 -> exp never overflows and
  the causal diagonal keeps denominators O(1)).

Softmax layout: scores are built transposed (scoresT[kpos, q]) so the
probs @ V matmul needs no transposes; kv chunks are processed in pairs
(one [128,512] bias-FMA + exp per pair); the denominator comes from
ones-stationary matmuls accumulated alongside and the per-q reciprocal is
broadcast across partitions with a K=1 outer-product matmul (f32r so it
runs at full speed without a cast). The ALiBi rel/mask tile depends only
on delta = j - 2g, so 16 distinct [128,256] tiles cover all (g, j).

Weights are pre-tiled on the host into SBUF partition-major layouts so
every weight DMA is fully contiguous.
"""
import math
import os

import numpy as np
import ml_dtypes

import concourse.bass as bass
from concourse import bacc
import concourse.mybir as mybir
from concourse.tile import TileContext
from concourse.bass_utils import run_bass_kernel_spmd

B, S, H, NH, HD = 2, 2048, 4096, 32, 128
NC = 8              # cores
TPC = 512           # tokens per core (256 per batch)
KC = H // 128       # 32 contraction chunks
GQ = 8              # 256-token q blocks per batch
SCALE = HD ** -0.5
# per-slot ALiBi reach (slot s = head octile): 45/slope maxed over octile
SLOT_D = [180.0, 1440.0, float("inf"), float("inf")]
USE_F32R_RB = False
bf16 = mybir.dt.bfloat16
f32 = mybir.dt.float32
f32r = mybir.dt.float32r
Exp = mybir.ActivationFunctionType.Exp
Copy = mybir.ActivationFunctionType.Copy
MULT = mybir.AluOpType.mult
ADD = mybir.AluOpType.add

_CACHE = {}
LAST_EXEC_NS = None


def _alibi_slopes(n_heads):
    closest_pow2 = 2 ** math.floor(math.log2(n_heads))
    base = 2.0 ** (-(2.0 ** -(math.log2(closest_pow2) - 3)))
    slopes = [base ** i for i in range(1, closest_pow2 + 1)]
    if closest_pow2 != n_heads:
        extra_base = 2.0 ** (-(2.0 ** -(math.log2(2 * closest_pow2) - 3)))
        n_extra = min(closest_pow2, n_heads - closest_pow2)
        slopes += [extra_base ** i for i in range(1, 2 * n_extra + 1, 2)]
    return np.asarray(slopes, dtype=np.float32)


def _j0(g, slot):
    d = SLOT_D[slot]
    if math.isinf(d):
        return 0
    return max(0, math.ceil((256 * g - 127 - d) / 128))


def _build_rel():
    # tile for delta = j - 2g at slice index (delta + 14): rel = 128*delta + p - f
    rel = np.empty((128, 16 * 256), np.float32)
    p = np.arange(128)[:, None]
    f = np.arange(256)[None, :]
    for idx in range(16):
        delta = idx - 14
        r = (128 * delta + p - f).astype(np.float32)
        r[128 * delta + p - f > 0] = -30000.0
        rel[:, 256 * idx:256 * (idx + 1)] = r
    return rel.astype(ml_dtypes.bfloat16)


def _build_nc():
    nc = bacc.Bacc(num_devices=NC)
    # host-pre-tiled layouts: every DMA below is contiguous in DRAM
    hsT = nc.declare_dram_parameter("hsT", [128, KC * TPC], bf16, isOutput=False)
    Wq_t = nc.declare_dram_parameter("Wq_t", [32, 128, KC * 128], bf16, isOutput=False)
    Wkv = nc.declare_dram_parameter("Wkv", [128, KC * 256], bf16, isOutput=False)
    Wo_t = nc.declare_dram_parameter("Wo_t", [8, 128, KC * 512], bf16, isOutput=False)
    rel = nc.declare_dram_parameter("rel", [128, 16 * 256], bf16, isOutput=False)
    slopes = nc.declare_dram_parameter("slopes", [128, 4], f32, isOutput=False)
    out = nc.declare_dram_parameter("out", [TPC, H], f32, isOutput=True)

    grp = [list(range(NC))]
    with TileContext(nc) as tc:
        with (
            tc.tile_pool(name="dram", bufs=1, space="DRAM") as dram,
            tc.tile_pool(name="const", bufs=1) as const,
            tc.tile_pool(name="psum", bufs=1, space="PSUM") as psum,
        ):
            kT_in = dram.tile([128, TPC], bf16)
            kT_ag = dram.tile([128 * NC, TPC], bf16, addr_space="Shared")
            v_in = dram.tile([TPC, 128], bf16)
            v_ag = dram.tile([TPC * NC, 128], bf16, addr_space="Shared")
            # q bounce buffers per (head-half hp, batch b)
            q_in = {(p_, b): dram.tile([H // 2, 256], bf16,
                                       name=f"q_in{p_}{b}")
                    for p_ in range(2) for b in range(2)}
            q_a2a = {(p_, b): dram.tile([H // 2, 256], bf16,
                                        name=f"q_a2a{p_}{b}")
                     for p_ in range(2) for b in range(2)}
            a_in = [dram.tile([H, 256], bf16, name=f"a_in{b}")
                    for b in range(2)]
            a_a2a = [dram.tile([H, 256], bf16, name=f"a_a2a{b}")
                     for b in range(2)]

            rel_sb = const.tile([128, 16 * 256], bf16)
            nc.sync.dma_start(out=rel_sb[:], in_=rel[:])
            slopes_sb = const.tile([128, 4], f32)
            nc.sync.dma_start(out=slopes_sb[:], in_=slopes[:])
            ones_col = const.tile([128, 1], bf16)
            nc.vector.memset(ones_col[:], 1.0)
            ones_row = const.tile([1, 128], f32r if USE_F32R_RB else bf16)
            nc.vector.memset(ones_row[:], 1.0)

            # ---------------- Phase 1: q/k/v projections -------------------
            with tc.tile_pool(name="ph1", bufs=1) as ph1:
                hsT_sb = ph1.tile([128, KC, TPC], bf16)
                nc.sync.dma_start(out=hsT_sb[:],
                                  in_=hsT.rearrange("p (k t) -> p k t", k=KC))
                Wkv_sb = ph1.tile([128, KC, 256], bf16)
                nc.sync.dma_start(out=Wkv_sb[:],
                                  in_=Wkv.rearrange("p (k c) -> p k c", k=KC))

                kT_ps = psum.tile([128, TPC], f32, tag="big", bufs=1)
                for k in range(KC):
                    nc.tensor.matmul(kT_ps[:], lhsT=Wkv_sb[:, k, 0:128],
                                     rhs=hsT_sb[:, k, :],
                                     start=(k == 0), stop=(k == KC - 1))
                kT_sb = ph1.tile([128, TPC], bf16)
                nc.vector.tensor_copy(out=kT_sb[:], in_=kT_ps[:])
                nc.sync.dma_start(out=kT_in[:], in_=kT_sb[:])

                for t4 in range(4):
                    v_ps = psum.tile([128, 128], f32, tag="big", bufs=1,
                                     name="v_ps")
                    for k in range(KC):
                        nc.tensor.matmul(
                            v_ps[:],
                            lhsT=hsT_sb[:, k, 128 * t4:128 * (t4 + 1)],
                            rhs=Wkv_sb[:, k, 128:256],
                            start=(k == 0), stop=(k == KC - 1))
                    v_sb = ph1.tile([128, 128], bf16, tag="v_sb", bufs=3,
                                    name="v_sb")
                    nc.vector.tensor_copy(out=v_sb[:], in_=v_ps[:])
                    nc.sync.dma_start(out=v_in[128 * t4:128 * (t4 + 1), :],
                                      in_=v_sb[:])

                # K/V AllGathers issue as soon as the tiny kv DMAs land,
                # overlapping the whole q projection below.
                nc.gpsimd.collective_compute(
                    "AllGather", mybir.AluOpType.bypass, replica_groups=grp,
                    ins=[kT_in[:]], outs=[kT_ag[:]])
                nc.gpsimd.collective_compute(
                    "AllGather", mybir.AluOpType.bypass, replica_groups=grp,
                    ins=[v_in[:]], outs=[v_ag[:]])

                # slots 0-1 heads first so their AllToAll can fire early
                dq_order = [d for d in range(32) if d // 8 < 2] + \
                           [d for d in range(32) if d // 8 >= 2]
                for dq in dq_order:
                    wq_sb = ph1.tile([128, KC, 128], bf16, tag="wq", bufs=3,
                                     name="wq_sb")
                    nc.sync.dma_start(
                        out=wq_sb[:],
                        in_=Wq_t[dq].rearrange("p (k m) -> p k m", k=KC))
                    q_ps = psum.tile([128, TPC], f32, tag="big", bufs=1,
                                     name="q_ps")
                    for k in range(KC):
                        nc.tensor.matmul(q_ps[:], lhsT=wq_sb[:, k, :],
                                         rhs=hsT_sb[:, k, :],
                                         start=(k == 0), stop=(k == KC - 1))
                    q_sb = ph1.tile([128, TPC], bf16, tag="qstage", bufs=3,
                                    name="q_sb")
                    nc.vector.tensor_scalar_mul(q_sb[:], q_ps[:], SCALE)
                    # head dq -> rank dq%8, slot dq//8 (round-robin heads)
                    hp_, sl_ = (dq // 8) // 2, (dq // 8) % 2
                    row = 256 * (dq % 8) + 128 * sl_
                    for b in range(2):
                        nc.sync.dma_start(
                            out=q_in[hp_, b][row:row + 128, :],
                            in_=q_sb[:, 256 * b:256 * (b + 1)])
                    if dq == dq_order[15]:  # slots 0-1 done -> ship half-A
                        for b in range(2):
                            nc.gpsimd.collective_compute(
                                "AllToAll", mybir.AluOpType.bypass,
                                replica_groups=grp,
                                ins=[q_in[0, b][:]], outs=[q_a2a[0, b][:]])
            for b in range(2):
                nc.gpsimd.collective_compute(
                    "AllToAll", mybir.AluOpType.bypass, replica_groups=grp,
                    ins=[q_in[1, b][:]], outs=[q_a2a[1, b][:]])

            # ---------------- Phases 3+4: attention & output projection ----
            # my slot-s head: global head = c + 8s; slope = slopes_sb[:, s]
            with (tc.tile_pool(name="attn", bufs=1) as attn,
                  tc.tile_pool(name="ph4", bufs=1) as ph4):
                kT_b, v_b, qT, aT = {}, {}, {}, {}
                for b in range(B):
                    t = attn.tile([128, 8, 256], bf16, name=f"kT_{b}")
                    nc.sync.dma_start(
                        out=t[:],
                        in_=kT_ag.rearrange("(r p) (b t) -> b p r t",
                                            p=128, b=2)[b])
                    kT_b[b] = t
                    t = attn.tile([128, 8, 2, 128], bf16, name=f"v_{b}")
                    for u in range(2):
                        nc.sync.dma_start(
                            out=t[:, :, u, :],
                            in_=v_ag.rearrange("(r b u p) d -> b p r u d",
                                               b=2, u=2, p=128)[b][:, :, u, :])
                    v_b[b] = t

                def kT_chunk(b, j):
                    return kT_b[b][:, j // 2, 128 * (j % 2):128 * (j % 2 + 1)]

                for b in range(B):
                    for s in range(4):
                        t = attn.tile([128, 8, 256], bf16, tag="qT", bufs=8,
                                      name=f"qT_{b}_{s}")
                        nc.sync.dma_start(
                            out=t[:],
                            in_=q_a2a[s // 2, b].rearrange(
                                "(j s p) t -> s p j t", s=2, p=128)[s % 2])
                        qT[b, s] = t
                        aT[b, s] = attn.tile([128, 8, 256], bf16, tag="aT",
                                             bufs=4, name=f"aT_{b}_{s}")

                def attn_unit(b, hp, g):
                    slots = (2 * hp, 2 * hp + 1)
                    nch = 2 * (g + 1)
                    j0 = [_j0(g, s) for s in slots]
                    at = [psum.tile([128, 256], f32, tag="at", bufs=2,
                                    name=f"at{hi}") for hi in range(2)]
                    den = [psum.tile([1, 256], f32, tag="den", bufs=2,
                                     name=f"den{hi}") for hi in range(2)]

                    def pv_den(hi, expp, base, ja, jb):
                        for ji, j in enumerate((ja, jb)):
                            if j is None:
                                continue
                            e_sl = expp[:, base + 256 * ji:base + 256 * (ji + 1)]
                            nc.tensor.matmul(
                                at[hi][:], lhsT=v_b[b][:, j // 2, j % 2, :],
                                rhs=e_sl, start=(j == j0[hi]),
                                stop=(j == nch - 1))
                            nc.tensor.matmul(
                                den[hi][:], lhsT=ones_col[:], rhs=e_sl,
                                start=(j == j0[hi]), stop=(j == nch - 1))

                    if j0[0] == j0[1]:
                        # identical chunk ranges: one wide exp per pair
                        js = list(range(j0[0], nch))
                        pairs = [(js[i], js[i + 1] if i + 1 < len(js)
                                  else None)
                                 for i in range(0, len(js), 2)]
                        for ja, jb in pairs:
                            w = 512 if jb is not None else 256
                            tmp = attn.tile([128, 1024], f32, tag="stt",
                                            bufs=2, name="tmp")
                            expp = attn.tile([128, 1024], bf16, tag="exp",
                                             bufs=4, name="expp")
                            for hi in range(2):
                                s2 = psum.tile([128, 512], f32, tag="s2",
                                               bufs=2, name="s2")
                                nc.tensor.matmul(
                                    s2[:, 0:256], lhsT=kT_chunk(b, ja),
                                    rhs=qT[b, slots[hi]][:, g, :],
                                    start=True, stop=True)
                                if jb is not None:
                                    nc.tensor.matmul(
                                        s2[:, 256:512], lhsT=kT_chunk(b, jb),
                                        rhs=qT[b, slots[hi]][:, g, :],
                                        start=True, stop=True)
                                d0 = 256 * (ja - 2 * g + 14)
                                nc.vector.scalar_tensor_tensor(
                                    out=tmp[:, w * hi:w * hi + w],
                                    in0=rel_sb[:, d0:d0 + w],
                                    scalar=slopes_sb[:, slots[hi]:
                                                     slots[hi] + 1],
                                    in1=s2[:, 0:w], op0=MULT, op1=ADD)
                            nc.scalar.activation(expp[:, 0:2 * w],
                                                 tmp[:, 0:2 * w], Exp)
                            for hi in range(2):
                                pv_den(hi, expp, w * hi, ja, jb)
                    else:
                        for hi in range(2):
                            sl = slots[hi]
                            js = list(range(j0[hi], nch))
                            pairs = [(js[i], js[i + 1] if i + 1 < len(js)
                                      else None)
                                     for i in range(0, len(js), 2)]
                            for ja, jb in pairs:
                                w = 512 if jb is not None else 256
                                s2 = psum.tile([128, 512], f32, tag="s2",
                                               bufs=2, name="s2")
                                nc.tensor.matmul(
                                    s2[:, 0:256], lhsT=kT_chunk(b, ja),
                                    rhs=qT[b, sl][:, g, :], start=True,
                                    stop=True)
                                if jb is not None:
                                    nc.tensor.matmul(
                                        s2[:, 256:512], lhsT=kT_chunk(b, jb),
                                        rhs=qT[b, sl][:, g, :], start=True,
                                        stop=True)
                                tmp = attn.tile([128, 512], f32, tag="stt2",
                                                bufs=2, name="tmp2")
                                expp = attn.tile([128, 512], bf16,
                                                 tag="exp2", bufs=4,
                                                 name="expp2")
                                d0 = 256 * (ja - 2 * g + 14)
                                nc.vector.scalar_tensor_tensor(
                                    out=tmp[:, 0:w],
                                    in0=rel_sb[:, d0:d0 + w],
                                    scalar=slopes_sb[:, sl:sl + 1],
                                    in1=s2[:, 0:w], op0=MULT, op1=ADD)
                                nc.scalar.activation(expp[:, 0:w],
                                                     tmp[:, 0:w], Exp)
                                pv_den(hi, expp, 0, ja, jb)
                    rec = attn.tile([1, 512], f32, tag="rec", bufs=2,
                                    name="rec")
                    for hi in range(2):
                        nc.vector.reciprocal_approx_fast(
                            out=rec[0:1, 256 * hi:256 * (hi + 1)],
                            in_=den[hi][:])
                    rb_ps = psum.tile([128, 512], f32, tag="rb", bufs=1,
                                      name="rb_ps")
                    if USE_F32R_RB:
                        nc.tensor.matmul(rb_ps[:], lhsT=ones_row[:],
                                         rhs=rec[:].bitcast(f32r),
                                         start=True, stop=True)
                    else:
                        rec_bf = attn.tile([1, 512], bf16, tag="rec_bf",
                                           bufs=2, name="rec_bf")
                        nc.vector.tensor_copy(out=rec_bf[:], in_=rec[:])
                        nc.tensor.matmul(rb_ps[:], lhsT=ones_row[:],
                                         rhs=rec_bf[:], start=True, stop=True)
                    rb_sb = attn.tile([128, 512], f32, tag="rbs", bufs=2,
                                      name="rb_sb")
                    nc.scalar.activation(rb_sb[:], rb_ps[:], Copy)
                    for hi in range(2):
                        nc.vector.tensor_tensor(
                            out=aT[b, slots[hi]][:, g, :], in0=at[hi][:],
                            in1=rb_sb[:, 256 * hi:256 * (hi + 1)], op=MULT)

                def ship_attnT(b):
                    for s in range(4):
                        nc.sync.dma_start(
                            out=a_in[b].rearrange("(j s p) t -> s p j t",
                                                  s=4, p=128)[s],
                            in_=aT[b, s][:])
                    nc.gpsimd.collective_compute(
                        "AllToAll", mybir.AluOpType.bypass,
                        replica_groups=grp,
                        ins=[a_in[b][:]], outs=[a_a2a[b][:]])

                att_sb = {}

                def load_att_sb(b):
                    att_sb[b] = ph4.tile([128, KC, 256], bf16,
                                         name=f"att_sb{b}")
                    nc.sync.dma_start(
                        out=att_sb[b][:],
                        in_=a_a2a[b].rearrange("(l p) t -> p l t", p=128))

                wo_tiles = {}

                def outproj_unit(b, n8, th):
                    if n8 not in wo_tiles:
                        w = ph4.tile([128, KC, 512], bf16, tag="wo", bufs=2,
                                     name="wo_sb")
                        nc.sync.dma_start(
                            out=w[:],
                            in_=Wo_t[n8].rearrange("p (k n) -> p k n", k=KC))
                        wo_tiles[n8] = w
                    wo_sb = wo_tiles[n8]
                    o_ps = psum.tile([128, 512], f32, tag="big", bufs=1,
                                     name="o_ps")
                    for k in range(KC):
                        l = 4 * (k % 8) + (k // 8)
                        nc.tensor.matmul(
                            o_ps[:],
                            lhsT=att_sb[b][:, l, 128 * th:128 * (th + 1)],
                            rhs=wo_sb[:, k, :],
                            start=(k == 0), stop=(k == KC - 1))
                    o_sb = ph4.tile([128, 512], f32, tag="ostage", bufs=2,
                                    name="o_sb")
                    nc.vector.tensor_copy(out=o_sb[:], in_=o_ps[:])
                    r0 = 256 * b + 128 * th
                    nc.sync.dma_start(
                        out=out[r0:r0 + 128, 512 * n8:512 * (n8 + 1)],
                        in_=o_sb[:])

                # batch 0 attention (hp0 can start during the q projection)
                for hp in range(2):
                    for g in range(GQ):
                        attn_unit(0, hp, g)
                ship_attnT(0)
                # batch 1 attention interleaved with batch 0 output proj
                load_att_sb(0)
                units_a = [(hp, g) for hp in range(2) for g in range(GQ)]
                units_o = [(n8, th) for n8 in range(8) for th in range(2)]
                for i in range(16):
                    attn_unit(1, *units_a[i])
                    outproj_unit(0, *units_o[i])
                ship_attnT(1)
                load_att_sb(1)
                wo_tiles.clear()
                for n8 in range(8):
                    for th in range(2):
                        outproj_unit(1, n8, th)
    nc.finalize()
    return nc


def kernel(hidden_states, Wq, Wkv, Wo):
    global LAST_EXEC_NS
    bf = ml_dtypes.bfloat16
    hs = np.asarray(hidden_states, dtype=np.float32)
    Wq = np.asarray(Wq, dtype=np.float32)
    Wkv_np = np.asarray(Wkv, dtype=np.float32)
    Wo = np.asarray(Wo, dtype=np.float32)

    Wq_t = np.ascontiguousarray(
        Wq.reshape(KC, 128, 32, 128).transpose(2, 1, 0, 3)
        .reshape(32, 128, KC * 128)).astype(bf)
    Wo_t = np.ascontiguousarray(
        Wo.reshape(KC, 128, 8, 512).transpose(2, 1, 0, 3)
        .reshape(8, 128, KC * 512)).astype(bf)
    Wkv_t = np.ascontiguousarray(
        Wkv_np.reshape(KC, 128, 256).transpose(1, 0, 2)
        .reshape(128, KC * 256)).astype(bf)
    rel = _build_rel()
    slopes = _alibi_slopes(NH)

    in_maps = []
    for c in range(NC):
        blk = np.concatenate([hs[0, 256 * c:256 * (c + 1)],
                              hs[1, 256 * c:256 * (c + 1)]], axis=0)
        hsT_c = np.ascontiguousarray(
            blk.T.reshape(KC, 128, TPC).transpose(1, 0, 2)
            .reshape(128, KC * TPC)).astype(bf)
        my_heads = [c + 8 * s for s in range(4)]
        slopes_c = np.ascontiguousarray(
            np.broadcast_to(slopes[my_heads][None, :], (128, 4)))
        in_maps.append({
            "hsT": hsT_c, "Wq_t": Wq_t, "Wkv": Wkv_t, "Wo_t": Wo_t,
            "rel": rel, "slopes": slopes_c,
        })

    if "nc" not in _CACHE:
        _CACHE["nc"] = _build_nc()
    nc = _CACHE["nc"]
    trace = bool(int(os.environ.get("BASS_KERNEL_TRACE", "0")))
    res = run_bass_kernel_spmd(nc, in_maps, core_ids=list(range(NC)),
                               trace=trace)
    LAST_EXEC_NS = res.exec_time_ns
    out_full = np.empty((B, S, H), np.float32)
    for c in range(NC):
        oc = res.results[c]["out"]
        out_full[0, 256 * c:256 * (c + 1)] = oc[0:256]
        out_full[1, 256 * c:256 * (c + 1)] = oc[256:512]
    return out_full



# revision 24
# speedup vs baseline: 1.2860x; 1.2860x over previous
"""MQA attention with ALiBi + causal mask on 8 TRN2 NeuronCores.

Problem: hidden_states [2,2048,4096] @ Wq -> 32 query heads of 128; single
KV head via Wkv; scores + ALiBi bias + causal mask; softmax; @ Wo.

Distribution (avoids the 64 MiB AllReduce of plain head-TP):
- Core c owns tokens [256c, 256(c+1)) of BOTH batches for all projections
  (output rows disjoint -> host concatenates). Attention is head-sharded
  round-robin (core c gets heads {c, c+8, c+16, c+24}) so every core's
  causal + ALiBi-cutoff workload is identical. The two shardings are
  bridged by small bf16 AllToAlls of qT / attnT plus AllGathers of the
  tiny single-head K/V.
- ALiBi distance cutoff: a kv chunk whose distance exceeds 45/slope
  contributes < 1e-13 of the softmax mass and is skipped. Per-slot
  (head-octile) uniform bounds keep the SPMD program identical per core.
- All matmuls in bf16 (rel-err budget 2e-2); softmax in f32 without
  max-subtraction (scores are O(10) -> exp never overflows and the causal
  diagonal keeps denominators O(1)).

Softmax layout: scores are built transposed (scoresT[kpos, q]) so the
probs @ V matmul needs no transposes. The ALiBi bias + causal mask are
applied MULTIPLICATIVELY after exp: expp = exp(s) * E_rel where
E_rel = exp(slope * rel) is host-precomputed in bf16 (0 where masked) —
this replaces a f32 DVE FMA with a half-cost bf16 multiply. The
denominator uses an all-ones [128,128] stationary so every matmul keeps
the full 128-row PE config (a 1-row stationary forces an array
reconfig/drain on every PV<->den transition, measured ~90ns each), and
the [128,q] broadcast denominator comes out of PSUM directly — no
broadcast matmul needed; a [128,512] DVE reciprocal feeds the final
normalize. Score/PV/den pairs are software-pipelined one-behind so the
PE never waits on ACT exp / DVE multiply.

Schedule: kv proj (chunked loads start matmuls ~immediately) -> K/V
AllGather (overlaps everything) -> q proj (per-head, PSUM double-
buffered, scale folded into Wq host-side, half-A AllToAll fires at head
16) -> attention batch 0 -> ship aT(0) -> attention batch 1 (hides
A2A(0) + att_sb(0) load + first wo prefetches) -> ship aT(1) ->
outproj batch 0 (hides A2A(1)) -> outproj batch 1. Wo streams in 16
[128, KC*256] blocks, bufs=4, two-ahead prefetch; batch 1 walks blocks
in [12..15, 11..0] order so the last four blocks are reused hot and
rotation WARs line up with consumption.

Weights are pre-tiled on the host into SBUF partition-major layouts so
every weight DMA is fully contiguous.
"""
import math
import os

import numpy as np
import ml_dtypes

import concourse.bass as bass
from concourse import bacc
import concourse.mybir as mybir
from concourse.tile import TileContext
from concourse.bass_utils import run_bass_kernel_spmd

B, S, H, NH, HD = 2, 2048, 4096, 32, 128
NC = 8              # cores
TPC = 512           # tokens per core (256 per batch)
KC = H // 128       # 32 contraction chunks
GQ = 8              # 256-token q blocks per batch
NB = 16             # 256-col output blocks
SCALE = HD ** -0.5
# per-slot ALiBi reach (slot s = head octile): 18/slope at the octile's
# smallest slope. A dropped chunk contributes < e^-18 of the diagonal's
# softmax mass (measured worst-head rel err 5e-9 vs full attention).
SLOT_D = [72.0, 288.0, 1152.0, float("inf")]
bf16 = mybir.dt.bfloat16
f32 = mybir.dt.float32
Exp = mybir.ActivationFunctionType.Exp
MULT = mybir.AluOpType.mult

_CACHE = {}
LAST_EXEC_NS = None


def _alibi_slopes(n_heads):
    closest_pow2 = 2 ** math.floor(math.log2(n_heads))
    base = 2.0 ** (-(2.0 ** -(math.log2(closest_pow2) - 3)))
    slopes = [base ** i for i in range(1, closest_pow2 + 1)]
    if closest_pow2 != n_heads:
        extra_base = 2.0 ** (-(2.0 ** -(math.log2(2 * closest_pow2) - 3)))
        n_extra = min(closest_pow2, n_heads - closest_pow2)
        slopes += [extra_base ** i for i in range(1, 2 * n_extra + 1, 2)]
    return np.asarray(slopes, dtype=np.float32)


def _j0(g, slot):
    d = SLOT_D[slot]
    if math.isinf(d):
        return 0
    return max(0, math.ceil((256 * g - 127 - d) / 128))


def _build_erel(slopes4):
    # E_rel[s][:, 256*(delta+14):...] = exp(slope_s * (128*delta + p - f)),
    # 0 where causally masked (rel > 0)
    p = np.arange(128)[:, None]
    f = np.arange(256)[None, :]
    rel = np.empty((128, 16 * 256), np.float32)
    for idx in range(16):
        delta = idx - 14
        r = (128 * delta + p - f).astype(np.float32)
        r[128 * delta + p - f > 0] = -np.inf
        rel[:, 256 * idx:256 * (idx + 1)] = r
    E = np.empty((128, 4 * 4096), np.float32)
    for s in range(4):
        E[:, 4096 * s:4096 * (s + 1)] = np.exp(float(slopes4[s]) * rel)
    return E.astype(ml_dtypes.bfloat16)


def _build_nc():
    nc = bacc.Bacc(num_devices=NC)
    # host-pre-tiled layouts: every DMA below is contiguous in DRAM
    hsT = nc.declare_dram_parameter("hsT", [128, KC * TPC], bf16, isOutput=False)
    Wq_t = nc.declare_dram_parameter("Wq_t", [32, 128, KC * 128], bf16, isOutput=False)
    Wkv = nc.declare_dram_parameter("Wkv", [128, KC * 256], bf16, isOutput=False)
    Wo_t = nc.declare_dram_parameter("Wo_t", [NB, 128, KC * 256], bf16, isOutput=False)
    erel = nc.declare_dram_parameter("erel", [128, 4 * 4096], bf16, isOutput=False)
    out = nc.declare_dram_parameter("out", [TPC, H], f32, isOutput=True)

    grp = [list(range(NC))]
    with TileContext(nc) as tc:
        with (
            tc.tile_pool(name="dram", bufs=1, space="DRAM") as dram,
            tc.tile_pool(name="const", bufs=1) as const,
            tc.tile_pool(name="early", bufs=1) as early,
            tc.tile_pool(name="psum", bufs=1, space="PSUM") as psum,
        ):
            kT_in = dram.tile([128, TPC], bf16)
            kT_ag = dram.tile([128 * NC, TPC], bf16, addr_space="Shared")
            v_in = dram.tile([TPC, 128], bf16)
            v_ag = dram.tile([TPC * NC, 128], bf16, addr_space="Shared")
            # q bounce buffers per (head-half hp, batch b)
            q_in = {(p_, b): dram.tile([H // 2, 256], bf16,
                                       name=f"q_in{p_}{b}")
                    for p_ in range(2) for b in range(2)}
            q_a2a = {(p_, b): dram.tile([H // 2, 256], bf16,
                                        name=f"q_a2a{p_}{b}")
                     for p_ in range(2) for b in range(2)}
            a_in = [dram.tile([H, 256], bf16, name=f"a_in{b}")
                    for b in range(2)]
            a_a2a = [dram.tile([H, 256], bf16, name=f"a_a2a{b}")
                     for b in range(2)]

            erel_sb = const.tile([128, 4, 4096], bf16)
            ones_mat = const.tile([128, 128], bf16)
            nc.vector.memset(ones_mat[:], 1.0)
            # warm the ACT exp table so attention's first exp doesn't pay
            # the ~5us table-load + sync at phase boundary
            warm = const.tile([1, 4], f32)
            nc.scalar.activation(warm[0:1, 0:1], ones_mat[0:1, 0:1], Exp)

            # pre-created attention input tiles (disjoint from ph1's SBUF,
            # so their DMAs prefetch during the q projection)
            kT_b = {b: early.tile([128, 8, 256], bf16, name=f"kT_{b}")
                    for b in range(2)}
            v_b = {b: early.tile([128, 8, 2, 128], bf16, name=f"v_{b}")
                   for b in range(2)}
            qT = {(b, s): early.tile([128, 8, 256], bf16, tag="qT", bufs=4,
                                     name=f"qT_{b}_{s}")
                  for b in range(2) for s in range(4)}

            def load_qT(b, s):
                nc.sync.dma_start(
                    out=qT[b, s][:],
                    in_=q_a2a[s // 2, b].rearrange(
                        "(j s p) t -> s p j t", s=2, p=128)[s % 2])

            # ---------------- Phase 1: q/k/v projections -------------------
            with tc.tile_pool(name="ph1", bufs=1) as ph1:
                hsT_sb = ph1.tile([128, KC, TPC], bf16)
                Wkv_sb = ph1.tile([128, KC, 256], bf16)
                hsT_re = hsT.rearrange("p (k t) -> p k t", k=KC)
                Wkv_re = Wkv.rearrange("p (k c) -> p k c", k=KC)
                # slots 0-1 heads first so their AllToAll can fire early
                dq_order = [d for d in range(32) if d // 8 < 2] + \
                           [d for d in range(32) if d // 8 >= 2]
                wq_tiles = {}

                def load_wq(dq):
                    t = ph1.tile([128, KC, 128], bf16, tag="wq", bufs=4,
                                 name="wq_sb")
                    nc.sync.dma_start(
                        out=t[:],
                        in_=Wq_t[dq].rearrange("p (k m) -> p k m", k=KC))
                    wq_tiles[dq] = t

                # chunked loads: kT matmul k can fire as soon as chunk k
                # lands; first two Wq tiles slip in mid-stream so the q
                # projection starts the moment the kv projection drains
                for k in range(KC):
                    nc.sync.dma_start(out=Wkv_sb[:, k, :], in_=Wkv_re[:, k, :])
                    nc.sync.dma_start(out=hsT_sb[:, k, :], in_=hsT_re[:, k, :])
                    if k == 10:
                        load_wq(dq_order[0])
                    if k == 20:
                        load_wq(dq_order[1])

                kT_ps = psum.tile([128, 512], f32, tag="big", bufs=2,
                                  name="kT_ps")
                for k in range(KC):
                    nc.tensor.matmul(kT_ps[:], lhsT=Wkv_sb[:, k, 0:128],
                                     rhs=hsT_sb[:, k, :],
                                     start=(k == 0), stop=(k == KC - 1))
                kT_sb = ph1.tile([128, TPC], bf16)
                nc.vector.tensor_copy(out=kT_sb[:], in_=kT_ps[:])
                nc.sync.dma_start(out=kT_in[:], in_=kT_sb[:])

                for t4 in range(4):
                    v_ps = psum.tile([128, 512], f32, tag="big", bufs=2,
                                     name="v_ps")
                    for k in range(KC):
                        nc.tensor.matmul(
                            v_ps[:, 0:128],
                            lhsT=hsT_sb[:, k, 128 * t4:128 * (t4 + 1)],
                            rhs=Wkv_sb[:, k, 128:256],
                            start=(k == 0), stop=(k == KC - 1))
                    v_sb = ph1.tile([128, 128], bf16, tag="v_sb", bufs=3,
                                    name="v_sb")
                    nc.vector.tensor_copy(out=v_sb[:], in_=v_ps[:, 0:128])
                    nc.sync.dma_start(out=v_in[128 * t4:128 * (t4 + 1), :],
                                      in_=v_sb[:])

                # K/V AllGathers issue as soon as the tiny kv DMAs land,
                # overlapping the whole q projection below.
                nc.gpsimd.collective_compute(
                    "AllGather", mybir.AluOpType.bypass, replica_groups=grp,
                    ins=[kT_in[:]], outs=[kT_ag[:]])
                nc.gpsimd.collective_compute(
                    "AllGather", mybir.AluOpType.bypass, replica_groups=grp,
                    ins=[v_in[:]], outs=[v_ag[:]])

                for dq in dq_order:
                    if dq not in wq_tiles:
                        load_wq(dq)
                    wq_sb = wq_tiles[dq]
                    q_ps = psum.tile([128, 512], f32, tag="big", bufs=2,
                                     name="q_ps")
                    for k in range(KC):
                        nc.tensor.matmul(q_ps[:], lhsT=wq_sb[:, k, :],
                                         rhs=hsT_sb[:, k, :],
                                         start=(k == 0), stop=(k == KC - 1))
                    q_sb = ph1.tile([128, TPC], bf16, tag="qstage", bufs=4,
                                    name="q_sb")
                    # drain on ACT (idle all q-proj; keeps DVE off the
                    # critical PSUM-rotation path)
                    nc.scalar.activation(q_sb[:], q_ps[:],
                                         mybir.ActivationFunctionType.Copy)
                    # head dq -> rank dq%8, slot dq//8 (round-robin heads)
                    hp_, sl_ = (dq // 8) // 2, (dq // 8) % 2
                    row = 256 * (dq % 8) + 128 * sl_
                    for b in range(2):
                        nc.sync.dma_start(
                            out=q_in[hp_, b][row:row + 128, :],
                            in_=q_sb[:, 256 * b:256 * (b + 1)])
                    if dq == dq_order[15]:  # slots 0-1 done -> ship half-A
                        for b in range(2):
                            nc.gpsimd.collective_compute(
                                "AllToAll", mybir.AluOpType.bypass,
                                replica_groups=grp,
                                ins=[q_in[0, b][:]], outs=[q_a2a[0, b][:]])

                # collective-gated prefetches LAST: a DMA waiting on an
                # AllGather/AllToAll at the head of a hardware queue would
                # head-of-line-block any weight load queued behind it (the
                # 100us q-proj starvation seen under inter-core launch skew)
                nc.sync.dma_start(
                    out=erel_sb[:],
                    in_=erel.rearrange("p (s d) -> p s d", s=4))
                for b in range(B):
                    nc.sync.dma_start(
                        out=kT_b[b][:],
                        in_=kT_ag.rearrange("(r p) (b t) -> b p r t",
                                            p=128, b=2)[b])
                    for u in range(2):
                        nc.sync.dma_start(
                            out=v_b[b][:, :, u, :],
                            in_=v_ag.rearrange("(r b u p) d -> b p r u d",
                                               b=2, u=2, p=128)[b][:, :, u, :])
                load_qT(0, 0)
                load_qT(0, 1)
            for b in range(2):
                nc.gpsimd.collective_compute(
                    "AllToAll", mybir.AluOpType.bypass, replica_groups=grp,
                    ins=[q_in[1, b][:]], outs=[q_a2a[1, b][:]])

            # ---------------- Phases 3+4: attention & output projection ----
            # my slot-s head: global head = c + 8s
            with (tc.tile_pool(name="attn", bufs=1) as attn,
                  tc.tile_pool(name="ph4", bufs=1) as ph4):
                aT = {(b, s): attn.tile([128, 8, 256], bf16, tag="aT",
                                        bufs=4, name=f"aT_{b}_{s}")
                      for b in range(2) for s in range(4)}

                def kT_chunk(b, j):
                    return kT_b[b][:, j // 2, 128 * (j % 2):128 * (j % 2 + 1)]

                pend = {"pv": None, "tail": None}

                def drain_pend():
                    if pend["pv"] is not None:
                        pend["pv"]()
                        pend["pv"] = None
                    if pend["tail"] is not None:
                        pend["tail"]()
                        pend["tail"] = None

                def attn_unit(b, hp, g):
                    slots = (2 * hp, 2 * hp + 1)
                    nch = 2 * (g + 1)
                    # flat pair list: head 0's pairs then head 1's, so each
                    # head's PSUM accumulation group closes before the next
                    # opens
                    work = []
                    for hi in range(2):
                        js = list(range(_j0(g, slots[hi]), nch))
                        for i in range(0, len(js), 2):
                            work.append((hi, js[i],
                                         js[i + 1] if i + 1 < len(js)
                                         else None))
                    at_t = psum.tile([128, 512], f32, tag="at", bufs=2,
                                     name="at_t")
                    den = psum.tile([128, 512], f32, tag="den", bufs=2,
                                    name="den")
                    started = [False, False]

                    def make_pv(hi, ja, jb, expp):
                        def pv():
                            st = not started[hi]
                            started[hi] = True
                            for ji, j in enumerate((ja, jb)):
                                if j is None:
                                    continue
                                e_sl = expp[:, 256 * ji:256 * (ji + 1)]
                                nc.tensor.matmul(
                                    at_t[:, 256 * hi:256 * (hi + 1)],
                                    lhsT=v_b[b][:, j // 2, j % 2, :],
                                    rhs=e_sl, start=(st and ji == 0),
                                    stop=(j == nch - 1))
                            for ji, j in enumerate((ja, jb)):
                                if j is None:
                                    continue
                                e_sl = expp[:, 256 * ji:256 * (ji + 1)]
                                nc.tensor.matmul(
                                    den[:, 256 * hi:256 * (hi + 1)],
                                    lhsT=ones_mat[:], rhs=e_sl,
                                    start=(st and ji == 0),
                                    stop=(j == nch - 1))
                        return pv

                    for hi, ja, jb in work:
                        wdt = 512 if jb is not None else 256
                        s2 = psum.tile([128, 512], f32, tag="s2", bufs=2,
                                       name="s2")
                        nc.tensor.matmul(
                            s2[:, 0:256], lhsT=kT_chunk(b, ja),
                            rhs=qT[b, slots[hi]][:, g, :],
                            start=True, stop=True)
                        if jb is not None:
                            nc.tensor.matmul(
                                s2[:, 256:512], lhsT=kT_chunk(b, jb),
                                rhs=qT[b, slots[hi]][:, g, :],
                                start=True, stop=True)
                        drain_pend()
                        expp = attn.tile([128, 512], bf16, tag="exp", bufs=4,
                                         name="expp")
                        nc.scalar.activation(expp[:, 0:wdt], s2[:, 0:wdt],
                                             Exp)
                        expm = attn.tile([128, 512], bf16, tag="expm",
                                         bufs=4, name="expm")
                        c0 = 256 * (ja - 2 * g + 14)
                        nc.vector.tensor_tensor(
                            out=expm[:, 0:wdt], in0=expp[:, 0:wdt],
                            in1=erel_sb[:, slots[hi], c0:c0 + wdt], op=MULT)
                        pend["pv"] = make_pv(hi, ja, jb, expm)

                    def tail():
                        denr = attn.tile([128, 512], f32, tag="denr",
                                         bufs=2, name="denr")
                        nc.vector.reciprocal_approx_fast(out=denr[:],
                                                         in_=den[:])
                        for hi in range(2):
                            nc.vector.tensor_tensor(
                                out=aT[b, slots[hi]][:, g, :],
                                in0=at_t[:, 256 * hi:256 * (hi + 1)],
                                in1=denr[:, 256 * hi:256 * (hi + 1)],
                                op=MULT)
                    pend["tail"] = tail

                def ship_attnT(b):
                    for s in range(4):
                        nc.sync.dma_start(
                            out=a_in[b].rearrange("(j s p) t -> s p j t",
                                                  s=4, p=128)[s],
                            in_=aT[b, s][:])
                    nc.gpsimd.collective_compute(
                        "AllToAll", mybir.AluOpType.bypass,
                        replica_groups=grp,
                        ins=[a_in[b][:]], outs=[a_a2a[b][:]])

                att_sb = {}

                def load_att_sb(b):
                    att_sb[b] = ph4.tile([128, KC, 256], bf16, tag="attsb",
                                         bufs=2, name=f"att_sb{b}")
                    nc.sync.dma_start(
                        out=att_sb[b][:],
                        in_=a_a2a[b].rearrange("(l p) t -> p l t", p=128))

                wo_tiles = {}

                def preload_wo(nb, tiles=None):
                    tiles = wo_tiles if tiles is None else tiles
                    if nb in tiles or not 0 <= nb < NB:
                        return
                    w = ph4.tile([128, KC, 256], bf16, tag="wo", bufs=4,
                                 name="wo_sb")
                    nc.sync.dma_start(
                        out=w[:],
                        in_=Wo_t[nb].rearrange("p (k n) -> p k n", k=KC))
                    tiles[nb] = w

                def outproj_unit(b, nb, th, tiles=None):
                    wo_sb = (wo_tiles if tiles is None else tiles)[nb]
                    o_ps = psum.tile([128, 512], f32, tag="big", bufs=2,
                                     name="o_ps")
                    for k in range(KC):
                        l = 4 * (k % 8) + (k // 8)
                        nc.tensor.matmul(
                            o_ps[:, 0:256],
                            lhsT=att_sb[b][:, l, 128 * th:128 * (th + 1)],
                            rhs=wo_sb[:, k, :],
                            start=(k == 0), stop=(k == KC - 1))
                    # DVE drain (ACT is contended by attention exps in the
                    # overlap window); bufs=6 rides out ~25us out-DMA jams
                    # around the A2As
                    o_sb = ph4.tile([128, 256], f32, tag="ostage", bufs=6,
                                    name="o_sb")
                    nc.vector.tensor_copy(out=o_sb[:], in_=o_ps[:, 0:256])
                    r0 = 256 * b + 128 * th
                    nc.sync.dma_start(
                        out=out[r0:r0 + 128, 256 * nb:256 * (nb + 1)],
                        in_=o_sb[:])

                # batch 0 attention; wo blocks 0-1 prefetch under hp1
                for hp in range(2):
                    for g in range(GQ):
                        attn_unit(0, hp, g)
                        if hp == 0 and g == 0:
                            load_qT(0, 2)
                            load_qT(0, 3)
                        if hp == 1 and g == 0:
                            load_qT(1, 0)
                            load_qT(1, 1)
                        if hp == 1 and g in (1, 5):
                            preload_wo(g // 4)
                drain_pend()
                ship_attnT(0)
                load_att_sb(0)
                load_qT(1, 2)
                load_qT(1, 3)
                # batch 1 attention; wo blocks 2-3 prefetch underneath
                for hp in range(2):
                    for g in range(GQ):
                        attn_unit(1, hp, g)
                        if hp == 0 and g in (1, 3):
                            preload_wo(2 + g // 2)
                drain_pend()
                ship_attnT(1)
                # batch 0 outproj (hides A2A(1)); two-ahead wo prefetch
                for nb in range(NB):
                    preload_wo(nb + 4)
                    for th in range(2):
                        outproj_unit(0, nb, th)
                    if nb == 1:
                        load_att_sb(1)
                # batch 1: last four wo blocks still resident (they own the
                # four rotation slots); walk them first so fresh preloads'
                # WARs line up with consumption order
                wo_tiles1 = {nb: wo_tiles[nb] for nb in (12, 13, 14, 15)}
                order1 = [12, 13, 14, 15] + list(range(11, -1, -1))
                for i, nb in enumerate(order1):
                    if i + 4 < len(order1):
                        preload_wo(order1[i + 4], tiles=wo_tiles1)
                    for th in range(2):
                        outproj_unit(1, nb, th, tiles=wo_tiles1)
    nc.finalize()
    return nc


def kernel(hidden_states, Wq, Wkv, Wo):
    global LAST_EXEC_NS
    bf = ml_dtypes.bfloat16
    hs = np.asarray(hidden_states, dtype=np.float32)
    Wq = np.asarray(Wq, dtype=np.float32)
    Wkv_np = np.asarray(Wkv, dtype=np.float32)
    Wo = np.asarray(Wo, dtype=np.float32)

    Wq_t = np.ascontiguousarray(
        (Wq * SCALE).reshape(KC, 128, 32, 128).transpose(2, 1, 0, 3)
        .reshape(32, 128, KC * 128)).astype(bf)
    Wo_t = np.ascontiguousarray(
        Wo.reshape(KC, 128, NB, 256).transpose(2, 1, 0, 3)
        .reshape(NB, 128, KC * 256)).astype(bf)
    Wkv_t = np.ascontiguousarray(
        Wkv_np.reshape(KC, 128, 256).transpose(1, 0, 2)
        .reshape(128, KC * 256)).astype(bf)
    slopes = _alibi_slopes(NH)

    in_maps = []
    for c in range(NC):
        blk = np.concatenate([hs[0, 256 * c:256 * (c + 1)],
                              hs[1, 256 * c:256 * (c + 1)]], axis=0)
        hsT_c = np.ascontiguousarray(
            blk.T.reshape(KC, 128, TPC).transpose(1, 0, 2)
            .reshape(128, KC * TPC)).astype(bf)
        my_heads = [c + 8 * s for s in range(4)]
        erel_c = _build_erel(slopes[my_heads])
        in_maps.append({
            "hsT": hsT_c, "Wq_t": Wq_t, "Wkv": Wkv_t, "Wo_t": Wo_t,
            "erel": erel_c,
        })

    if "nc" not in _CACHE:
        _CACHE["nc"] = _build_nc()
    nc = _CACHE["nc"]
    trace = bool(int(os.environ.get("BASS_KERNEL_TRACE", "0")))
    res = run_bass_kernel_spmd(nc, in_maps, core_ids=list(range(NC)),
                               trace=trace)
    LAST_EXEC_NS = res.exec_time_ns
    out_full = np.empty((B, S, H), np.float32)
    for c in range(NC):
        oc = res.results[c]["out"]
        out_full[0, 256 * c:256 * (c + 1)] = oc[0:256]
        out_full[1, 256 * c:256 * (c + 1)] = oc[256:512]
    return out_full


# revision 31
# speedup vs baseline: 1.3401x; 1.0421x over previous
"""MQA attention with ALiBi + causal mask on 8 TRN2 NeuronCores.

Problem: hidden_states [2,2048,4096] @ Wq -> 32 query heads of 128; single
KV head via Wkv; scores + ALiBi bias + causal mask; softmax; @ Wo.

Distribution (avoids the 64 MiB AllReduce of plain head-TP):
- Core c owns tokens [256c, 256(c+1)) of BOTH batches for all projections
  (output rows disjoint -> host concatenates). Attention is head-sharded
  round-robin (core c gets heads {c, c+8, c+16, c+24}) so every core's
  causal + ALiBi-cutoff workload is identical. The two shardings are
  bridged by small bf16 AllToAlls of qT / attnT plus AllGathers of the
  tiny single-head K/V.
- ALiBi distance cutoff: a kv chunk whose distance exceeds 45/slope
  contributes < 1e-13 of the softmax mass and is skipped. Per-slot
  (head-octile) uniform bounds keep the SPMD program identical per core.
- All matmuls in bf16 (rel-err budget 2e-2); softmax in f32 without
  max-subtraction (scores are O(10) -> exp never overflows and the causal
  diagonal keeps denominators O(1)).

Softmax layout: scores are built transposed (scoresT[kpos, q]) so the
probs @ V matmul needs no transposes. The ALiBi bias + causal mask are
applied MULTIPLICATIVELY after exp: expp = exp(s) * E_rel where
E_rel = exp(slope * rel) is host-precomputed in bf16 (0 where masked) —
this replaces a f32 DVE FMA with a half-cost bf16 multiply. The
denominator uses an all-ones [128,128] stationary so every matmul keeps
the full 128-row PE config (a 1-row stationary forces an array
reconfig/drain on every PV<->den transition, measured ~90ns each), and
the [128,q] broadcast denominator comes out of PSUM directly — no
broadcast matmul needed; a [128,512] DVE reciprocal feeds the final
normalize. Score/PV/den pairs are software-pipelined one-behind so the
PE never waits on ACT exp / DVE multiply.

Schedule: kv proj (chunked loads start matmuls ~immediately) -> K/V
AllGather (overlaps everything) -> q proj (per-head, PSUM double-
buffered, scale folded into Wq host-side, half-A AllToAll fires at head
16) -> attention batch 0 -> ship aT(0) -> attention batch 1 (hides
A2A(0) + att_sb(0) load + first wo prefetches) -> ship aT(1) ->
outproj batch 0 (hides A2A(1)) -> outproj batch 1. Wo streams in 16
[128, KC*256] blocks, bufs=4, two-ahead prefetch; batch 1 walks blocks
in [12..15, 11..0] order so the last four blocks are reused hot and
rotation WARs line up with consumption.

Weights are pre-tiled on the host into SBUF partition-major layouts so
every weight DMA is fully contiguous.
"""
import math
import os

import numpy as np
import ml_dtypes

import concourse.bass as bass
from concourse import bacc
import concourse.mybir as mybir
from concourse.tile import TileContext
from concourse.bass_utils import run_bass_kernel_spmd

B, S, H, NH, HD = 2, 2048, 4096, 32, 128
NC = 8              # cores
TPC = 512           # tokens per core (256 per batch)
KC = H // 128       # 32 contraction chunks
GQ = 8              # 256-token q blocks per batch
NB = 16             # 256-col output blocks
SCALE = HD ** -0.5
# per-slot ALiBi reach (slot s = head octile): 18/slope at the octile's
# smallest slope. A dropped chunk contributes < e^-18 of the diagonal's
# softmax mass (measured worst-head rel err 5e-9 vs full attention).
SLOT_D = [72.0, 288.0, 1152.0, float("inf")]
bf16 = mybir.dt.bfloat16
f32 = mybir.dt.float32
Exp = mybir.ActivationFunctionType.Exp
MULT = mybir.AluOpType.mult

_CACHE = {}
LAST_EXEC_NS = None


def _alibi_slopes(n_heads):
    closest_pow2 = 2 ** math.floor(math.log2(n_heads))
    base = 2.0 ** (-(2.0 ** -(math.log2(closest_pow2) - 3)))
    slopes = [base ** i for i in range(1, closest_pow2 + 1)]
    if closest_pow2 != n_heads:
        extra_base = 2.0 ** (-(2.0 ** -(math.log2(2 * closest_pow2) - 3)))
        n_extra = min(closest_pow2, n_heads - closest_pow2)
        slopes += [extra_base ** i for i in range(1, 2 * n_extra + 1, 2)]
    return np.asarray(slopes, dtype=np.float32)


def _j0(g, slot):
    d = SLOT_D[slot]
    if math.isinf(d):
        return 0
    return max(0, math.ceil((256 * g - 127 - d) / 128))


def _build_erel(slopes4):
    # E_rel[s][:, 256*(delta+14):...] = exp(slope_s * (128*delta + p - f)),
    # 0 where causally masked (rel > 0)
    p = np.arange(128)[:, None]
    f = np.arange(256)[None, :]
    rel = np.empty((128, 16 * 256), np.float32)
    for idx in range(16):
        delta = idx - 14
        r = (128 * delta + p - f).astype(np.float32)
        r[128 * delta + p - f > 0] = -np.inf
        rel[:, 256 * idx:256 * (idx + 1)] = r
    E = np.empty((128, 4 * 4096), np.float32)
    for s in range(4):
        E[:, 4096 * s:4096 * (s + 1)] = np.exp(float(slopes4[s]) * rel)
    return E.astype(ml_dtypes.bfloat16)


def _build_nc():
    nc = bacc.Bacc(num_devices=NC)
    # host-pre-tiled layouts: every DMA below is contiguous in DRAM
    hsT = nc.declare_dram_parameter("hsT", [128, KC * TPC], bf16, isOutput=False)
    Wq_t = nc.declare_dram_parameter("Wq_t", [32, 128, KC * 128], bf16, isOutput=False)
    Wkv = nc.declare_dram_parameter("Wkv", [128, KC * 256], bf16, isOutput=False)
    Wo_t = nc.declare_dram_parameter("Wo_t", [NB, 128, KC * 256], bf16, isOutput=False)
    erel = nc.declare_dram_parameter("erel", [128, 4 * 4096], bf16, isOutput=False)
    out = nc.declare_dram_parameter("out", [TPC, H], f32, isOutput=True)

    grp = [list(range(NC))]
    with TileContext(nc) as tc:
        with (
            tc.tile_pool(name="dram", bufs=1, space="DRAM") as dram,
            tc.tile_pool(name="const", bufs=1) as const,
            tc.tile_pool(name="early", bufs=1) as early,
            tc.tile_pool(name="psum", bufs=1, space="PSUM") as psum,
        ):
            kT_in = dram.tile([128, TPC], bf16)
            kT_ag = dram.tile([128 * NC, TPC], bf16, addr_space="Shared")
            v_in = dram.tile([TPC, 128], bf16)
            v_ag = dram.tile([TPC * NC, 128], bf16, addr_space="Shared")
            # q bounce buffers per (head-half hp, batch b)
            q_in = {(p_, b): dram.tile([H // 2, 256], bf16,
                                       name=f"q_in{p_}{b}")
                    for p_ in range(2) for b in range(2)}
            q_a2a = {(p_, b): dram.tile([H // 2, 256], bf16,
                                        name=f"q_a2a{p_}{b}")
                     for p_ in range(2) for b in range(2)}
            a_in = [dram.tile([H, 256], bf16, name=f"a_in{b}")
                    for b in range(2)]
            a_a2a = [dram.tile([H, 256], bf16, name=f"a_a2a{b}")
                     for b in range(2)]

            erel_sb = const.tile([128, 4, 4096], bf16)
            ones_mat = const.tile([128, 128], bf16)
            nc.vector.memset(ones_mat[:], 1.0)
            # warm the ACT exp table so attention's first exp doesn't pay
            # the ~5us table-load + sync at phase boundary
            warm = const.tile([1, 4], f32)
            nc.scalar.activation(warm[0:1, 0:1], ones_mat[0:1, 0:1], Exp)

            # pre-created attention input tiles (disjoint from ph1's SBUF,
            # so their DMAs prefetch during the q projection)
            kT_b = {b: early.tile([128, 8, 256], bf16, name=f"kT_{b}")
                    for b in range(2)}
            v_b = {b: early.tile([128, 8, 2, 128], bf16, name=f"v_{b}")
                   for b in range(2)}
            qT = {(b, s): early.tile([128, 8, 256], bf16, tag="qT", bufs=4,
                                     name=f"qT_{b}_{s}")
                  for b in range(2) for s in range(4)}

            def load_qT(b, s):
                nc.sync.dma_start(
                    out=qT[b, s][:],
                    in_=q_a2a[s // 2, b].rearrange(
                        "(j s p) t -> s p j t", s=2, p=128)[s % 2])

            # ---------------- Phase 1: q/k/v projections -------------------
            with tc.tile_pool(name="ph1", bufs=1) as ph1:
                hsT_sb = ph1.tile([128, KC, TPC], bf16)
                Wkv_sb = ph1.tile([128, KC, 256], bf16)
                hsT_re = hsT.rearrange("p (k t) -> p k t", k=KC)
                Wkv_re = Wkv.rearrange("p (k c) -> p k c", k=KC)
                # slots 2-3 (full-causal, most attention work) first so
                # their AllToAll fires at head 16 and attention leads with
                # them — the second A2A then hides under ~54us of hp1 work
                dq_order = [d for d in range(32) if d // 8 >= 2] + \
                           [d for d in range(32) if d // 8 < 2]
                wq_tiles = {}

                def load_wq(dq):
                    t = ph1.tile([128, KC, 128], bf16, tag="wq", bufs=4,
                                 name="wq_sb")
                    nc.sync.dma_start(
                        out=t[:],
                        in_=Wq_t[dq].rearrange("p (k m) -> p k m", k=KC))
                    wq_tiles[dq] = t

                # chunked loads: kT matmul k can fire as soon as chunk k
                # lands; first two Wq tiles slip in mid-stream so the q
                # projection starts the moment the kv projection drains
                for k in range(0, KC, 2):
                    nc.sync.dma_start(out=Wkv_sb[:, k:k + 2, :],
                                      in_=Wkv_re[:, k:k + 2, :])
                    nc.sync.dma_start(out=hsT_sb[:, k:k + 2, :],
                                      in_=hsT_re[:, k:k + 2, :])
                    if k == 10:
                        load_wq(dq_order[0])
                    if k == 20:
                        load_wq(dq_order[1])

                kT_ps = psum.tile([128, 512], f32, tag="big", bufs=2,
                                  name="kT_ps")
                for k in range(KC):
                    nc.tensor.matmul(kT_ps[:], lhsT=Wkv_sb[:, k, 0:128],
                                     rhs=hsT_sb[:, k, :],
                                     start=(k == 0), stop=(k == KC - 1))
                kT_sb = ph1.tile([128, TPC], bf16)
                nc.vector.tensor_copy(out=kT_sb[:], in_=kT_ps[:])
                nc.sync.dma_start(out=kT_in[:], in_=kT_sb[:])

                for t4 in range(4):
                    v_ps = psum.tile([128, 512], f32, tag="big", bufs=2,
                                     name="v_ps")
                    for k in range(KC):
                        nc.tensor.matmul(
                            v_ps[:, 0:128],
                            lhsT=hsT_sb[:, k, 128 * t4:128 * (t4 + 1)],
                            rhs=Wkv_sb[:, k, 128:256],
                            start=(k == 0), stop=(k == KC - 1))
                    v_sb = ph1.tile([128, 128], bf16, tag="v_sb", bufs=3,
                                    name="v_sb")
                    nc.vector.tensor_copy(out=v_sb[:], in_=v_ps[:, 0:128])
                    nc.sync.dma_start(out=v_in[128 * t4:128 * (t4 + 1), :],
                                      in_=v_sb[:])

                # K/V AllGathers issue as soon as the tiny kv DMAs land,
                # overlapping the whole q projection below.
                nc.gpsimd.collective_compute(
                    "AllGather", mybir.AluOpType.bypass, replica_groups=grp,
                    ins=[kT_in[:]], outs=[kT_ag[:]])
                nc.gpsimd.collective_compute(
                    "AllGather", mybir.AluOpType.bypass, replica_groups=grp,
                    ins=[v_in[:]], outs=[v_ag[:]])

                for dq in dq_order:
                    if dq not in wq_tiles:
                        load_wq(dq)
                    wq_sb = wq_tiles[dq]
                    q_ps = psum.tile([128, 512], f32, tag="big", bufs=2,
                                     name="q_ps")
                    for k in range(KC):
                        nc.tensor.matmul(q_ps[:], lhsT=wq_sb[:, k, :],
                                         rhs=hsT_sb[:, k, :],
                                         start=(k == 0), stop=(k == KC - 1))
                    q_sb = ph1.tile([128, TPC], bf16, tag="qstage", bufs=6,
                                    name="q_sb")
                    # alternate drains across DVE/ACT (both idle in q-proj)
                    # so neither engine's queue gates the PSUM rotation
                    if dq % 2 == 0:
                        nc.vector.tensor_copy(out=q_sb[:], in_=q_ps[:])
                    else:
                        nc.scalar.activation(
                            q_sb[:], q_ps[:],
                            mybir.ActivationFunctionType.Copy)
                    # head dq -> rank dq%8, slot dq//8 (round-robin heads)
                    hp_, sl_ = (dq // 8) // 2, (dq // 8) % 2
                    row = 256 * (dq % 8) + 128 * sl_
                    for b in range(2):
                        nc.sync.dma_start(
                            out=q_in[hp_, b][row:row + 128, :],
                            in_=q_sb[:, 256 * b:256 * (b + 1)])
                    if dq == dq_order[15]:  # slots 2-3 done -> ship hp1 half
                        for b in range(2):
                            nc.gpsimd.collective_compute(
                                "AllToAll", mybir.AluOpType.bypass,
                                replica_groups=grp,
                                ins=[q_in[1, b][:]], outs=[q_a2a[1, b][:]])

                # collective-gated prefetches LAST: a DMA waiting on an
                # AllGather/AllToAll at the head of a hardware queue would
                # head-of-line-block any weight load queued behind it (the
                # 100us q-proj starvation seen under inter-core launch skew)
                nc.sync.dma_start(
                    out=erel_sb[:],
                    in_=erel.rearrange("p (s d) -> p s d", s=4))
                for b in range(B):
                    nc.sync.dma_start(
                        out=kT_b[b][:],
                        in_=kT_ag.rearrange("(r p) (b t) -> b p r t",
                                            p=128, b=2)[b])
                    for u in range(2):
                        nc.sync.dma_start(
                            out=v_b[b][:, :, u, :],
                            in_=v_ag.rearrange("(r b u p) d -> b p r u d",
                                               b=2, u=2, p=128)[b][:, :, u, :])
                load_qT(0, 2)
                load_qT(0, 3)
            for b in range(2):
                nc.gpsimd.collective_compute(
                    "AllToAll", mybir.AluOpType.bypass, replica_groups=grp,
                    ins=[q_in[0, b][:]], outs=[q_a2a[0, b][:]])

            # ---------------- Phases 3+4: attention & output projection ----
            # my slot-s head: global head = c + 8s
            with (tc.tile_pool(name="attn", bufs=1) as attn,
                  tc.tile_pool(name="ph4", bufs=1) as ph4):
                aT = {(b, s): attn.tile([128, 8, 256], bf16, tag="aT",
                                        bufs=4, name=f"aT_{b}_{s}")
                      for b in range(2) for s in range(4)}

                def kT_chunk(b, j):
                    return kT_b[b][:, j // 2, 128 * (j % 2):128 * (j % 2 + 1)]

                pend = {"pv": None, "tail": None}

                def drain_pend():
                    if pend["pv"] is not None:
                        pend["pv"]()
                        pend["pv"] = None
                    if pend["tail"] is not None:
                        pend["tail"]()
                        pend["tail"] = None

                def attn_unit(b, hp, g):
                    slots = (2 * hp, 2 * hp + 1)
                    nch = 2 * (g + 1)
                    # flat pair list: head 0's pairs then head 1's, so each
                    # head's PSUM accumulation group closes before the next
                    # opens
                    work = []
                    for hi in range(2):
                        js = list(range(_j0(g, slots[hi]), nch))
                        for i in range(0, len(js), 2):
                            work.append((hi, js[i],
                                         js[i + 1] if i + 1 < len(js)
                                         else None))
                    at_t = psum.tile([128, 512], f32, tag="at", bufs=2,
                                     name="at_t")
                    den = psum.tile([128, 512], f32, tag="den", bufs=2,
                                    name="den")
                    started = [False, False]

                    def make_pv(hi, ja, jb, expp):
                        def pv():
                            st = not started[hi]
                            started[hi] = True
                            for ji, j in enumerate((ja, jb)):
                                if j is None:
                                    continue
                                e_sl = expp[:, 256 * ji:256 * (ji + 1)]
                                nc.tensor.matmul(
                                    at_t[:, 256 * hi:256 * (hi + 1)],
                                    lhsT=v_b[b][:, j // 2, j % 2, :],
                                    rhs=e_sl, start=(st and ji == 0),
                                    stop=(j == nch - 1))
                            for ji, j in enumerate((ja, jb)):
                                if j is None:
                                    continue
                                e_sl = expp[:, 256 * ji:256 * (ji + 1)]
                                nc.tensor.matmul(
                                    den[:, 256 * hi:256 * (hi + 1)],
                                    lhsT=ones_mat[:], rhs=e_sl,
                                    start=(st and ji == 0),
                                    stop=(j == nch - 1))
                        return pv

                    for hi, ja, jb in work:
                        wdt = 512 if jb is not None else 256
                        s2 = psum.tile([128, 512], f32, tag="s2", bufs=2,
                                       name="s2")
                        nc.tensor.matmul(
                            s2[:, 0:256], lhsT=kT_chunk(b, ja),
                            rhs=qT[b, slots[hi]][:, g, :],
                            start=True, stop=True)
                        if jb is not None:
                            nc.tensor.matmul(
                                s2[:, 256:512], lhsT=kT_chunk(b, jb),
                                rhs=qT[b, slots[hi]][:, g, :],
                                start=True, stop=True)
                        drain_pend()
                        expp = attn.tile([128, 512], bf16, tag="exp", bufs=4,
                                         name="expp")
                        nc.scalar.activation(expp[:, 0:wdt], s2[:, 0:wdt],
                                             Exp)
                        expm = attn.tile([128, 512], bf16, tag="expm",
                                         bufs=4, name="expm")
                        c0 = 256 * (ja - 2 * g + 14)
                        nc.vector.tensor_tensor(
                            out=expm[:, 0:wdt], in0=expp[:, 0:wdt],
                            in1=erel_sb[:, slots[hi], c0:c0 + wdt], op=MULT)
                        pend["pv"] = make_pv(hi, ja, jb, expm)

                    def tail():
                        denr = attn.tile([128, 512], f32, tag="denr",
                                         bufs=2, name="denr")
                        nc.vector.reciprocal_approx_fast(out=denr[:],
                                                         in_=den[:])
                        for hi in range(2):
                            nc.vector.tensor_tensor(
                                out=aT[b, slots[hi]][:, g, :],
                                in0=at_t[:, 256 * hi:256 * (hi + 1)],
                                in1=denr[:, 256 * hi:256 * (hi + 1)],
                                op=MULT)
                    pend["tail"] = tail

                def ship_attnT(b):
                    for s in range(4):
                        nc.sync.dma_start(
                            out=a_in[b].rearrange("(j s p) t -> s p j t",
                                                  s=4, p=128)[s],
                            in_=aT[b, s][:])
                    nc.gpsimd.collective_compute(
                        "AllToAll", mybir.AluOpType.bypass,
                        replica_groups=grp,
                        ins=[a_in[b][:]], outs=[a_a2a[b][:]])

                att_sb = {}

                def load_att_sb(b):
                    att_sb[b] = ph4.tile([128, KC, 256], bf16, tag="attsb",
                                         bufs=2, name=f"att_sb{b}")
                    nc.sync.dma_start(
                        out=att_sb[b][:],
                        in_=a_a2a[b].rearrange("(l p) t -> p l t", p=128))

                wo_tiles = {}

                def preload_wo(nb, tiles=None):
                    tiles = wo_tiles if tiles is None else tiles
                    if nb in tiles or not 0 <= nb < NB:
                        return
                    w = ph4.tile([128, KC, 256], bf16, tag="wo", bufs=4,
                                 name="wo_sb")
                    nc.sync.dma_start(
                        out=w[:],
                        in_=Wo_t[nb].rearrange("p (k n) -> p k n", k=KC))
                    tiles[nb] = w

                def outproj_unit(b, nb, th, tiles=None):
                    wo_sb = (wo_tiles if tiles is None else tiles)[nb]
                    o_ps = psum.tile([128, 512], f32, tag="big", bufs=2,
                                     name="o_ps")
                    for k in range(KC):
                        l = 4 * (k % 8) + (k // 8)
                        nc.tensor.matmul(
                            o_ps[:, 0:256],
                            lhsT=att_sb[b][:, l, 128 * th:128 * (th + 1)],
                            rhs=wo_sb[:, k, :],
                            start=(k == 0), stop=(k == KC - 1))
                    # alternate drains DVE/ACT so neither engine's backlog
                    # in the attention-overlap window gates the PSUM
                    # rotation; bufs=6 rides out out-DMA jams near the A2As
                    o_sb = ph4.tile([128, 256], f32, tag="ostage", bufs=6,
                                    name="o_sb")
                    if th == 0:
                        nc.vector.tensor_copy(out=o_sb[:], in_=o_ps[:, 0:256])
                    else:
                        nc.scalar.activation(
                            o_sb[:], o_ps[:, 0:256],
                            mybir.ActivationFunctionType.Copy)
                    r0 = 256 * b + 128 * th
                    nc.sync.dma_start(
                        out=out[r0:r0 + 128, 256 * nb:256 * (nb + 1)],
                        in_=o_sb[:])

                # batch 0 attention, hp1 (slots 2-3, shipped first) leading;
                # wo blocks 0-1 prefetch underneath
                for hp in (1, 0):
                    for g in range(GQ):
                        attn_unit(0, hp, g)
                        if hp == 1 and g == 0:
                            load_qT(0, 0)
                            load_qT(0, 1)
                        if hp == 1 and g in (2, 5):
                            preload_wo(g // 4)
                        if hp == 0 and g == 0:
                            load_qT(1, 2)
                            load_qT(1, 3)
                drain_pend()
                ship_attnT(0)
                load_att_sb(0)
                load_qT(1, 0)
                load_qT(1, 1)
                # batch 1 attention; wo blocks 2-3 prefetch underneath
                for hp in (1, 0):
                    for g in range(GQ):
                        attn_unit(1, hp, g)
                        if hp == 1 and g in (1, 3):
                            preload_wo(2 + g // 2)
                drain_pend()
                ship_attnT(1)
                # batch 0 outproj (hides A2A(1)); two-ahead wo prefetch
                for nb in range(NB):
                    preload_wo(nb + 4)
                    for th in range(2):
                        outproj_unit(0, nb, th)
                    if nb == 1:
                        load_att_sb(1)
                # batch 1: last four wo blocks still resident (they own the
                # four rotation slots); walk them first so fresh preloads'
                # WARs line up with consumption order
                wo_tiles1 = {nb: wo_tiles[nb] for nb in (12, 13, 14, 15)}
                order1 = [12, 13, 14, 15] + list(range(11, -1, -1))
                for i, nb in enumerate(order1):
                    if i + 4 < len(order1):
                        preload_wo(order1[i + 4], tiles=wo_tiles1)
                    for th in range(2):
                        outproj_unit(1, nb, th, tiles=wo_tiles1)
    nc.finalize()
    return nc


def kernel(hidden_states, Wq, Wkv, Wo):
    global LAST_EXEC_NS
    bf = ml_dtypes.bfloat16
    hs = np.asarray(hidden_states, dtype=np.float32)
    Wq = np.asarray(Wq, dtype=np.float32)
    Wkv_np = np.asarray(Wkv, dtype=np.float32)
    Wo = np.asarray(Wo, dtype=np.float32)

    Wq_t = np.ascontiguousarray(
        (Wq * SCALE).reshape(KC, 128, 32, 128).transpose(2, 1, 0, 3)
        .reshape(32, 128, KC * 128)).astype(bf)
    Wo_t = np.ascontiguousarray(
        Wo.reshape(KC, 128, NB, 256).transpose(2, 1, 0, 3)
        .reshape(NB, 128, KC * 256)).astype(bf)
    Wkv_t = np.ascontiguousarray(
        Wkv_np.reshape(KC, 128, 256).transpose(1, 0, 2)
        .reshape(128, KC * 256)).astype(bf)
    slopes = _alibi_slopes(NH)

    in_maps = []
    for c in range(NC):
        blk = np.concatenate([hs[0, 256 * c:256 * (c + 1)],
                              hs[1, 256 * c:256 * (c + 1)]], axis=0)
        hsT_c = np.ascontiguousarray(
            blk.T.reshape(KC, 128, TPC).transpose(1, 0, 2)
            .reshape(128, KC * TPC)).astype(bf)
        my_heads = [c + 8 * s for s in range(4)]
        erel_c = _build_erel(slopes[my_heads])
        in_maps.append({
            "hsT": hsT_c, "Wq_t": Wq_t, "Wkv": Wkv_t, "Wo_t": Wo_t,
            "erel": erel_c,
        })

    if "nc" not in _CACHE:
        _CACHE["nc"] = _build_nc()
    nc = _CACHE["nc"]
    trace = bool(int(os.environ.get("BASS_KERNEL_TRACE", "0")))
    res = run_bass_kernel_spmd(nc, in_maps, core_ids=list(range(NC)),
                               trace=trace)
    LAST_EXEC_NS = res.exec_time_ns
    out_full = np.empty((B, S, H), np.float32)
    for c in range(NC):
        oc = res.results[c]["out"]
        out_full[0, 256 * c:256 * (c + 1)] = oc[0:256]
        out_full[1, 256 * c:256 * (c + 1)] = oc[256:512]
    return out_full
